# revision 2
# baseline (speedup 1.0000x reference)
"""Causal multi-head attention on 8 Trainium2 NeuronCores — v2.

Problem: x[2,4096,512], W_q/W_k/W_v/W_proj[512,512], b_proj[512]
  q,k,v = x @ W.T split into 8 heads of 64; causal softmax(q k^T / 8) v;
  out = attn @ W_proj.T + b_proj.

Sharding: 16 (batch, head) pairs over 8 cores -> each core gets one batch
and a pair of adjacent heads.  Per-core output projection against the
matching 128-row slice of W_proj^T gives a partial [4096, 512]; the host
sums 4 partials per batch and adds the bias.

v2 vs v1: the ScalarE exp stream is the hard floor (~143us/core), so the
kernel software-pipelines everything else around it: projections for
chunk qc+1 and normalize/out-proj for chunk qc-1 are emitted interleaved
into chunk qc's attention stream; k-blocks are processed diagonal-first
so mask-muls don't sit on the accumulation tail; v/exp/mask/attnT run in
bf16; softmax normalization uses a K=2 outer-product matmul broadcast.
"""

import numpy as np

B, S, D, H = 2, 4096, 512, 8
DH = 64
QCHUNK = 512
SCALE = 1.0 / np.sqrt(DH)

_CACHE = {}


def _build(s=S, repeats=1):
    from contextlib import ExitStack

    import concourse.mybir as mybir
    import concourse.tile as tile
    from concourse import bacc

    f32 = mybir.dt.float32
    f32r = mybir.dt.float32r
    bf16 = mybir.dt.bfloat16

    nc = bacc.Bacc("TRN2")
    d = {
        "xT": nc.dram_tensor("xT", [128, 4 * s], f32r, kind="ExternalInput"),
        "wqT": nc.dram_tensor("wqT", [128, D], f32r, kind="ExternalInput"),
        "wkT": nc.dram_tensor("wkT", [128, D], f32r, kind="ExternalInput"),
        "wvT": nc.dram_tensor("wvT", [128, D], bf16, kind="ExternalInput"),
        "wvTf": nc.dram_tensor("wvTf", [128, D], f32r, kind="ExternalInput"),
        "wpT": nc.dram_tensor("wpT", [128, D], bf16, kind="ExternalInput"),
        "mask_in": nc.dram_tensor("mask_in", [128, 2048], bf16,
                                  kind="ExternalInput"),
        "ones_in": nc.dram_tensor("ones_in", [1, 128], f32r,
                                  kind="ExternalInput"),
        "out_p": nc.dram_tensor("out_p", [s, D], f32, kind="ExternalOutput"),
    }

    with ExitStack() as ctx:
        tc = ctx.enter_context(tile.TileContext(nc))
        P = {
            "consts": ctx.enter_context(tc.tile_pool(name="consts", bufs=1)),
            "big": ctx.enter_context(tc.tile_pool(name="big", bufs=1)),
            "expool": ctx.enter_context(tc.tile_pool(name="expool", bufs=6)),
            "npool": ctx.enter_context(tc.tile_pool(name="npool", bufs=2)),
            "outpool": ctx.enter_context(tc.tile_pool(name="outpool", bufs=2)),
            "scps": ctx.enter_context(
                tc.tile_pool(name="scps", bufs=2, space="PSUM")),
            "accps": ctx.enter_context(
                tc.tile_pool(name="accps", bufs=1, space="PSUM")),
            "mmps": ctx.enter_context(
                tc.tile_pool(name="mmps", bufs=2, space="PSUM")),
        }
        for _rep in range(repeats):
            _emit_body(nc, P, d, s)

    nc.compile()
    return nc


def _emit_body(nc, P, d, s):
    from collections import deque

    import concourse.mybir as mybir

    f32 = mybir.dt.float32
    f32r = mybir.dt.float32r
    bf16 = mybir.dt.bfloat16
    EXP = mybir.ActivationFunctionType.Exp

    nqc = s // QCHUNK
    nkbs = s // 128
    ndc = D // 128

    consts, big = P["consts"], P["big"]
    expool, npool, outpool = P["expool"], P["npool"], P["outpool"]
    scps, accps, mmps = P["scps"], P["accps"], P["mmps"]

    wq = consts.tile([128, D], f32r, name="wq", tag="wq")
    wk = consts.tile([128, D], f32r, name="wk", tag="wk")
    wvb = consts.tile([128, D], bf16, name="wvb", tag="wvb")
    wvf = consts.tile([128, D], f32r, name="wvf", tag="wvf")
    wpb = consts.tile([128, D], bf16, name="wpb", tag="wpb")
    masks = consts.tile([128, 2048], bf16, name="masks", tag="masks")
    ones1 = consts.tile([1, 128], f32r, name="ones1", tag="ones1")

    xTa = big.tile([128, ndc * s], f32r, name="xTa", tag="xTa")
    xba = big.tile([128, ndc * s], bf16, name="xba", tag="xba")
    xT = [xTa[:, c * s:(c + 1) * s] for c in range(ndc)]
    xb = [xba[:, c * s:(c + 1) * s] for c in range(ndc)]
    qT = big.tile([128, s], f32r, name="qT", tag="qT")
    kT = big.tile([128, s], f32r, name="kT", tag="kT")
    v65 = [big.tile([128, 65 * nkbs], bf16, name=f"v65_{h}", tag=f"v65_{h}") for h in range(2)]
    attnT = big.tile([128, s], bf16, name="attnT", tag="attnT")

    # ---- prologue: DMAs (ordered by urgency) + on-device bf16 cast ----
    nc.sync.dma_start(out=xT[0], in_=d["xT"][:, 0:s])
    nc.sync.dma_start(out=wq, in_=d["wqT"].ap())
    nc.sync.dma_start(out=wk, in_=d["wkT"].ap())
    for c in range(1, ndc):
        nc.sync.dma_start(out=xT[c], in_=d["xT"][:, c * s:(c + 1) * s])
    nc.sync.dma_start(out=wvb, in_=d["wvT"].ap())
    nc.sync.dma_start(out=wvf, in_=d["wvTf"].ap())
    nc.sync.dma_start(out=wpb, in_=d["wpT"].ap())
    nc.sync.dma_start(out=masks, in_=d["mask_in"].ap())
    nc.sync.dma_start(out=ones1, in_=d["ones_in"].ap())
    for h in range(2):
        ones_ap = v65[h].rearrange("p (k c) -> p k c", c=65)[:, :, 64]
        nc.gpsimd.memset(ones_ap, 1.0)
    for c in range(ndc):
        nc.gpsimd.tensor_copy(xb[c], xT[c].bitcast(f32))

    # ---- aux-op closures ----
    def proj_closures(qc):
        qlo = qc * QCHUNK
        qs = slice(qlo, qlo + QCHUNK)
        cl = []

        def qk_proj(w_sb, dst):
            def run():
                ps = mmps.tile([128, QCHUNK], f32, name="mmq", tag="mm")
                for c in range(ndc):
                    nc.tensor.matmul(ps,
                                     lhsT=w_sb[:, c * 128:(c + 1) * 128],
                                     rhs=xT[c][:, qs],
                                     start=(c == 0), stop=(c == ndc - 1))
                nc.vector.tensor_copy(dst[:, qs], ps)
            return run

        cl.append(qk_proj(wq, qT))
        cl.append(qk_proj(wk, kT))
        for j in range(4):
            kb = qc * 4 + j

            def vproj(kb=kb, qc=qc):
                ks = slice(kb * 128, (kb + 1) * 128)
                vp = mmps.tile([128, 128], f32, name="mmv", tag="mm")
                xv, wv_ = (xT, wvf) if qc < 2 else (xb, wvb)
                for c in range(ndc):
                    nc.tensor.matmul(vp,
                                     lhsT=xv[c][:, ks],
                                     rhs=wv_[:, c * 128:(c + 1) * 128],
                                     start=(c == 0), stop=(c == ndc - 1))
                for h in range(2):
                    nc.vector.tensor_copy(v65[h][:, kb * 65:kb * 65 + 64],
                                          vp[:, h * 64:(h + 1) * 64])
            cl.append(vproj)
        return cl

    def phase3_closures(qc, dens):
        qlo = qc * QCHUNK
        qs = slice(qlo, qlo + QCHUNK)
        cl = []

        def norm():
            for h in range(2):
                rinv = npool.tile([1, QCHUNK], f32r, name=f"rinv{h}",
                                  tag=f"rinv{h}")
                with nc.allow_low_precision(reason="softmax denom in f32r"):
                    nc.vector.reciprocal(rinv, dens[h])
                bc = mmps.tile([64, QCHUNK], f32, name=f"bc{h}", tag="mm")
                nc.tensor.matmul(bc, lhsT=ones1[:, 0:64], rhs=rinv,
                                 start=True, stop=True)
                hsl = slice(h * 64, (h + 1) * 64)
                nc.vector.tensor_mul(attnT[hsl, qs], attnT[hsl, qs], bc)
        cl.append(norm)

        ot = outpool.tile([128, 4 * D], f32, name="ot", tag="ot")

        for j in range(4):
            qb = qc * 4 + j

            def pp_out(qb=qb, j=j):
                pp = mmps.tile([128, D], f32, name="mmp", tag="mm")
                nc.tensor.matmul(pp,
                                 lhsT=attnT[:, qb * 128:(qb + 1) * 128],
                                 rhs=wpb, start=True, stop=True)
                nc.vector.tensor_copy(ot[:, j * D:(j + 1) * D], pp)
            cl.append(pp_out)

        def out_dma():
            dst = d["out_p"][qlo:qlo + QCHUNK, :].rearrange(
                "(b p) c -> p b c", p=128)
            nc.sync.dma_start(out=dst, in_=ot)
        cl.append(out_dma)
        return cl

    # ---- main software-pipelined loop ----
    pend_proj = deque(proj_closures(0))
    pend_p3 = deque()
    for qc in range(nqc):
        # this chunk's q/k/v must be fully emitted before its stream
        while pend_proj:
            pend_proj.popleft()()
        if qc + 1 < nqc:
            pend_proj.extend(proj_closures(qc + 1))

        qlo = qc * QCHUNK
        qs = slice(qlo, qlo + QCHUNK)
        diag = [qc * 4 + j for j in range(4)]
        old = list(range(qc * 4))
        kbs = diag + old
        pairs = [(kbs[i], kbs[i + 1]) for i in range(0, len(kbs), 2)]
        total_kb = len(kbs)

        acc = [accps.tile([65, QCHUNK], f32, name=f"acc{h}", tag=f"acc{h}") for h in range(2)]
        n_acc = [0, 0]
        for kb0, kb1 in pairs:
            for h in range(2):
                hsl = slice(h * 64, (h + 1) * 64)
                sc = scps.tile([128, 1024], f32, name="sc", tag="sc")
                for j, kb in enumerate((kb0, kb1)):
                    nc.tensor.matmul(sc[:, j * 512:(j + 1) * 512],
                                     lhsT=kT[hsl, kb * 128:(kb + 1) * 128],
                                     rhs=qT[hsl, qs],
                                     start=True, stop=True)
                ex = expool.tile([128, 1024], bf16, name="ex", tag="ex")
                nc.scalar.activation(ex, sc, EXP, scale=float(SCALE))
                if kb0 >= qc * 4:  # diagonal pair: zero where k > q
                    r0 = kb0 - qc * 4
                    nc.vector.tensor_mul(
                        ex, ex, masks[:, r0 * 512:(r0 + 2) * 512])
                for j, kb in enumerate((kb0, kb1)):
                    n_acc[h] += 1
                    nc.tensor.matmul(acc[h],
                                     lhsT=v65[h][:, kb * 65:(kb + 1) * 65],
                                     rhs=ex[:, j * 512:(j + 1) * 512],
                                     start=(n_acc[h] == 1),
                                     stop=(n_acc[h] == total_kb))
                # interleave one pending aux op per stream unit
                if pend_proj:
                    pend_proj.popleft()()
                elif pend_p3:
                    pend_p3.popleft()()

        # stream end: drain acc -> SBUF so acc slots free up quickly
        dens = [npool.tile([1, QCHUNK], f32, name=f"den{h}", tag=f"den{h}")
                for h in range(2)]
        for h in range(2):
            nc.vector.tensor_copy(attnT[h * 64:(h + 1) * 64, qs],
                                  acc[h][0:64, :])
            nc.vector.tensor_copy(dens[h], acc[h][64:65, :])
        pend_p3.extend(phase3_closures(qc, dens))

    while pend_p3:
        pend_p3.popleft()()


def _in_maps(x, W_q, W_k, W_v, W_proj):
    import ml_dtypes
    bf16 = ml_dtypes.bfloat16

    mask = np.zeros((128, 2048), dtype=np.float32)
    for r in range(4):
        for p in range(128):
            lo = p + 128 * r
            if lo < 512:
                mask[p, r * 512 + lo:(r + 1) * 512] = 1.0
    mask = mask.astype(bf16)
    ones1 = np.ones((1, 128), dtype=np.float32)

    maps = []
    for c in range(8):
        b, hp = c // 4, c % 4
        cols = slice(hp * 128, (hp + 1) * 128)
        xt = x[b].T  # [512, 4096]
        xt_blk = np.concatenate([xt[c * 128:(c + 1) * 128, :]
                                 for c in range(4)], axis=1)  # [128, 4*4096]
        def arrange(Wslice):  # [128 out, 512 in] -> [128, 512] lhsT blocks
            return np.concatenate([Wslice[:, c * 128:(c + 1) * 128].T
                                   for c in range(4)], axis=1)
        maps.append({
            "xT": np.ascontiguousarray(xt_blk),
            "wqT": np.ascontiguousarray(arrange(W_q[cols, :])),
            "wkT": np.ascontiguousarray(arrange(W_k[cols, :])),
            "wvT": np.ascontiguousarray(arrange(W_v[cols, :])).astype(bf16),
            "wvTf": np.ascontiguousarray(arrange(W_v[cols, :])),
            "wpT": np.ascontiguousarray(W_proj[:, cols].T).astype(bf16),
            "mask_in": mask,
            "ones_in": ones1,
        })
    return maps


def kernel(x, W_q, W_k, W_v, W_proj, b_proj, _trace=False):
    from concourse.bass_utils import run_bass_kernel_spmd

    x = np.asarray(x, dtype=np.float32)
    W_q = np.asarray(W_q, dtype=np.float32)
    W_k = np.asarray(W_k, dtype=np.float32)
    W_v = np.asarray(W_v, dtype=np.float32)
    W_proj = np.asarray(W_proj, dtype=np.float32)
    b_proj = np.asarray(b_proj, dtype=np.float32)

    if "nc" not in _CACHE:
        _CACHE["nc"] = _build()
    nc = _CACHE["nc"]

    res = run_bass_kernel_spmd(nc, _in_maps(x, W_q, W_k, W_v, W_proj),
                               core_ids=list(range(8)), trace=_trace)
    out = np.empty((B, S, D), dtype=np.float32)
    for b in range(B):
        acc = res.results[4 * b]["out_p"].astype(np.float32)
        for j in range(1, 4):
            acc = acc + res.results[4 * b + j]["out_p"]
        out[b] = acc + b_proj
    if _trace:
        _CACHE["last_trace"] = res
    return out


# revision 4
# speedup vs baseline: 1.0504x; 1.0504x over previous
"""Causal multi-head attention on 8 Trainium2 NeuronCores — v3.

Problem: x[2,4096,512], W_q/W_k/W_v/W_proj[512,512], b_proj[512]
  q,k,v = x @ W.T split into 8 heads of 64; causal softmax(q k^T / 8) v;
  out = attn @ W_proj.T + b_proj.

Sharding: 16 (batch, head) pairs over 8 cores -> each core gets one batch
and a pair of adjacent heads.  Per-core output projection against the
matching 128-row slice of W_proj^T gives a partial [4096, 512]; the host
sums 4 partials per batch and adds the bias.

Design: everything software-pipelines around the attention stream:
projections for chunk qc+1 and normalize/out-proj for chunk qc-1 are
emitted interleaved into chunk qc's attention stream; k-blocks run
diagonal-first so mask-muls don't sit on the accumulation tail; all
matmul operands are bf16 (fp32 PSUM accumulation; measured L2 rel err
~6e-3 vs the fp32 reference); softmax normalization uses K=1
outer-product matmul broadcasts of the reciprocal denominator row.
"""

import numpy as np

B, S, D, H = 2, 4096, 512, 8
DH = 64
QCHUNK = 512
SCALE = 1.0 / np.sqrt(DH)

_CACHE = {}


def _build(s=S, repeats=1):
    from contextlib import ExitStack

    import concourse.mybir as mybir
    import concourse.tile as tile
    from concourse import bacc

    f32 = mybir.dt.float32
    f32r = mybir.dt.float32r
    bf16 = mybir.dt.bfloat16

    nc = bacc.Bacc("TRN2")
    d = {
        "xT": nc.dram_tensor("xT", [128, 4 * s], bf16, kind="ExternalInput"),
        "wqT": nc.dram_tensor("wqT", [128, D], bf16, kind="ExternalInput"),
        "wkT": nc.dram_tensor("wkT", [128, D], bf16, kind="ExternalInput"),
        "wvT": nc.dram_tensor("wvT", [128, D], bf16, kind="ExternalInput"),
        "wpT": nc.dram_tensor("wpT", [128, D], bf16, kind="ExternalInput"),
        "mask_in": nc.dram_tensor("mask_in", [128, 2048], bf16,
                                  kind="ExternalInput"),
        "ones_in": nc.dram_tensor("ones_in", [1, 128], bf16,
                                  kind="ExternalInput"),
        "out_p": nc.dram_tensor("out_p", [s, D], f32, kind="ExternalOutput"),
    }

    with ExitStack() as ctx:
        tc = ctx.enter_context(tile.TileContext(nc))
        P = {
            "consts": ctx.enter_context(tc.tile_pool(name="consts", bufs=1)),
            "big": ctx.enter_context(tc.tile_pool(name="big", bufs=1)),
            "expool": ctx.enter_context(tc.tile_pool(name="expool", bufs=6)),
            "npool": ctx.enter_context(tc.tile_pool(name="npool", bufs=2)),
            "outpool": ctx.enter_context(tc.tile_pool(name="outpool", bufs=2)),
            "scps": ctx.enter_context(
                tc.tile_pool(name="scps", bufs=2, space="PSUM")),
            "accps": ctx.enter_context(
                tc.tile_pool(name="accps", bufs=1, space="PSUM")),
            "mmps": ctx.enter_context(
                tc.tile_pool(name="mmps", bufs=2, space="PSUM")),
        }
        for _rep in range(repeats):
            _emit_body(nc, P, d, s)

    nc.compile()
    return nc


def _emit_body(nc, P, d, s):
    from collections import deque

    import concourse.mybir as mybir

    f32 = mybir.dt.float32
    f32r = mybir.dt.float32r
    bf16 = mybir.dt.bfloat16
    EXP = mybir.ActivationFunctionType.Exp

    nqc = s // QCHUNK
    nkbs = s // 128
    ndc = D // 128

    consts, big = P["consts"], P["big"]
    expool, npool, outpool = P["expool"], P["npool"], P["outpool"]
    scps, accps, mmps = P["scps"], P["accps"], P["mmps"]

    wq = consts.tile([128, D], bf16, name="wq", tag="wq")
    wk = consts.tile([128, D], bf16, name="wk", tag="wk")
    wvb = consts.tile([128, D], bf16, name="wvb", tag="wvb")
    wpb = consts.tile([128, D], bf16, name="wpb", tag="wpb")
    masks = consts.tile([128, 2048], bf16, name="masks", tag="masks")
    ones1 = consts.tile([1, 128], bf16, name="ones1", tag="ones1")

    xTa = big.tile([128, ndc * s], bf16, name="xTa", tag="xTa")
    xT = [xTa[:, c * s:(c + 1) * s] for c in range(ndc)]
    qT = big.tile([128, s], bf16, name="qT", tag="qT")
    kT = big.tile([128, s], bf16, name="kT", tag="kT")
    v65 = [big.tile([128, 65 * nkbs], bf16, name=f"v65_{h}", tag=f"v65_{h}") for h in range(2)]
    attnT = big.tile([128, s], bf16, name="attnT", tag="attnT")

    # ---- prologue: DMAs (ordered by urgency) + on-device bf16 cast ----
    nc.sync.dma_start(out=xT[0], in_=d["xT"][:, 0:s])
    nc.sync.dma_start(out=wq, in_=d["wqT"].ap())
    nc.sync.dma_start(out=wk, in_=d["wkT"].ap())
    for c in range(1, ndc):
        nc.sync.dma_start(out=xT[c], in_=d["xT"][:, c * s:(c + 1) * s])
    nc.sync.dma_start(out=wvb, in_=d["wvT"].ap())
    nc.sync.dma_start(out=wpb, in_=d["wpT"].ap())
    nc.sync.dma_start(out=masks, in_=d["mask_in"].ap())
    nc.sync.dma_start(out=ones1, in_=d["ones_in"].ap())
    for h in range(2):
        ones_ap = v65[h].rearrange("p (k c) -> p k c", c=65)[:, :, 64]
        nc.gpsimd.memset(ones_ap, 1.0)

    # ---- aux-op closures ----
    def proj_closures(qc):
        qlo = qc * QCHUNK
        qs = slice(qlo, qlo + QCHUNK)
        cl = []

        def qk_proj(w_sb, dst):
            def run():
                ps = mmps.tile([128, QCHUNK], f32, name="mmq", tag="mm")
                for c in range(ndc):
                    nc.tensor.matmul(ps,
                                     lhsT=w_sb[:, c * 128:(c + 1) * 128],
                                     rhs=xT[c][:, qs],
                                     start=(c == 0), stop=(c == ndc - 1))
                nc.vector.tensor_copy(dst[:, qs], ps)
            return run

        cl.append(qk_proj(wq, qT))
        cl.append(qk_proj(wk, kT))
        for j in range(4):
            kb = qc * 4 + j

            def vproj(kb=kb):
                ks = slice(kb * 128, (kb + 1) * 128)
                vp = mmps.tile([128, 128], f32, name="mmv", tag="mm")
                for c in range(ndc):
                    nc.tensor.matmul(vp,
                                     lhsT=xT[c][:, ks],
                                     rhs=wvb[:, c * 128:(c + 1) * 128],
                                     start=(c == 0), stop=(c == ndc - 1))
                for h in range(2):
                    nc.vector.tensor_copy(v65[h][:, kb * 65:kb * 65 + 64],
                                          vp[:, h * 64:(h + 1) * 64])
            cl.append(vproj)
        return cl

    def phase3_closures(qc, dens):
        qlo = qc * QCHUNK
        qs = slice(qlo, qlo + QCHUNK)
        cl = []

        def norm():
            for h in range(2):
                rinv = npool.tile([1, QCHUNK], bf16, name=f"rinv{h}",
                                  tag=f"rinv{h}")
                with nc.allow_low_precision(reason="softmax denom in bf16"):
                    nc.vector.reciprocal(rinv, dens[h])
                bc = mmps.tile([64, QCHUNK], f32, name=f"bc{h}", tag="mm")
                nc.tensor.matmul(bc, lhsT=ones1[:, 0:64], rhs=rinv,
                                 start=True, stop=True)
                hsl = slice(h * 64, (h + 1) * 64)
                nc.vector.tensor_mul(attnT[hsl, qs], attnT[hsl, qs], bc)
        cl.append(norm)

        ot = outpool.tile([128, 4 * D], f32, name="ot", tag="ot")

        for j in range(4):
            qb = qc * 4 + j

            def pp_out(qb=qb, j=j):
                pp = mmps.tile([128, D], f32, name="mmp", tag="mm")
                nc.tensor.matmul(pp,
                                 lhsT=attnT[:, qb * 128:(qb + 1) * 128],
                                 rhs=wpb, start=True, stop=True)
                nc.vector.tensor_copy(ot[:, j * D:(j + 1) * D], pp)
            cl.append(pp_out)

        def out_dma():
            dst = d["out_p"][qlo:qlo + QCHUNK, :].rearrange(
                "(b p) c -> p b c", p=128)
            nc.sync.dma_start(out=dst, in_=ot)
        cl.append(out_dma)
        return cl

    # ---- main software-pipelined loop ----
    pend_proj = deque(proj_closures(0))
    pend_p3 = deque()
    for qc in range(nqc):
        # this chunk's q/k/v must be fully emitted before its stream
        while pend_proj:
            pend_proj.popleft()()
        if qc + 1 < nqc:
            pend_proj.extend(proj_closures(qc + 1))

        qlo = qc * QCHUNK
        qs = slice(qlo, qlo + QCHUNK)
        diag = [qc * 4 + j for j in range(4)]
        old = list(range(qc * 4))
        kbs = diag + old
        pairs = [(kbs[i], kbs[i + 1]) for i in range(0, len(kbs), 2)]
        total_kb = len(kbs)

        acc = [accps.tile([65, QCHUNK], f32, name=f"acc{h}", tag=f"acc{h}") for h in range(2)]
        n_acc = [0, 0]
        for kb0, kb1 in pairs:
            for h in range(2):
                hsl = slice(h * 64, (h + 1) * 64)
                sc = scps.tile([128, 1024], f32, name="sc", tag="sc")
                for j, kb in enumerate((kb0, kb1)):
                    nc.tensor.matmul(sc[:, j * 512:(j + 1) * 512],
                                     lhsT=kT[hsl, kb * 128:(kb + 1) * 128],
                                     rhs=qT[hsl, qs],
                                     start=True, stop=True)
                ex = expool.tile([128, 1024], bf16, name="ex", tag="ex")
                nc.scalar.activation(ex, sc, EXP, scale=float(SCALE))
                if kb0 >= qc * 4:  # diagonal pair: zero where k > q
                    r0 = kb0 - qc * 4
                    nc.vector.tensor_mul(
                        ex, ex, masks[:, r0 * 512:(r0 + 2) * 512])
                for j, kb in enumerate((kb0, kb1)):
                    n_acc[h] += 1
                    nc.tensor.matmul(acc[h],
                                     lhsT=v65[h][:, kb * 65:(kb + 1) * 65],
                                     rhs=ex[:, j * 512:(j + 1) * 512],
                                     start=(n_acc[h] == 1),
                                     stop=(n_acc[h] == total_kb))
                # interleave one pending aux op per stream unit
                if pend_proj:
                    pend_proj.popleft()()
                elif pend_p3:
                    pend_p3.popleft()()

        # stream end: drain acc -> SBUF so acc slots free up quickly
        dens = [npool.tile([1, QCHUNK], f32, name=f"den{h}", tag=f"den{h}")
                for h in range(2)]
        for h in range(2):
            nc.vector.tensor_copy(attnT[h * 64:(h + 1) * 64, qs],
                                  acc[h][0:64, :])
            nc.vector.tensor_copy(dens[h], acc[h][64:65, :])
        pend_p3.extend(phase3_closures(qc, dens))

    while pend_p3:
        pend_p3.popleft()()


def _in_maps(x, W_q, W_k, W_v, W_proj):
    import ml_dtypes
    bf16 = ml_dtypes.bfloat16

    mask = np.zeros((128, 2048), dtype=np.float32)
    for r in range(4):
        for p in range(128):
            lo = p + 128 * r
            if lo < 512:
                mask[p, r * 512 + lo:(r + 1) * 512] = 1.0
    mask = mask.astype(bf16)
    ones1 = np.ones((1, 128), dtype=np.float32).astype(bf16)

    maps = []
    for c in range(8):
        b, hp = c // 4, c % 4
        cols = slice(hp * 128, (hp + 1) * 128)
        xt = x[b].T  # [512, 4096]
        xt_blk = np.concatenate([xt[c * 128:(c + 1) * 128, :]
                                 for c in range(4)], axis=1)  # [128, 4*4096]
        def arrange(Wslice):  # [128 out, 512 in] -> [128, 512] lhsT blocks
            return np.concatenate([Wslice[:, c * 128:(c + 1) * 128].T
                                   for c in range(4)], axis=1)
        maps.append({
            "xT": np.ascontiguousarray(xt_blk).astype(bf16),
            "wqT": np.ascontiguousarray(arrange(W_q[cols, :])).astype(bf16),
            "wkT": np.ascontiguousarray(arrange(W_k[cols, :])).astype(bf16),
            "wvT": np.ascontiguousarray(arrange(W_v[cols, :])).astype(bf16),
            "wpT": np.ascontiguousarray(W_proj[:, cols].T).astype(bf16),
            "mask_in": mask,
            "ones_in": ones1,
        })
    return maps


def kernel(x, W_q, W_k, W_v, W_proj, b_proj, _trace=False):
    from concourse.bass_utils import run_bass_kernel_spmd

    x = np.asarray(x, dtype=np.float32)
    W_q = np.asarray(W_q, dtype=np.float32)
    W_k = np.asarray(W_k, dtype=np.float32)
    W_v = np.asarray(W_v, dtype=np.float32)
    W_proj = np.asarray(W_proj, dtype=np.float32)
    b_proj = np.asarray(b_proj, dtype=np.float32)

    if "nc" not in _CACHE:
        _CACHE["nc"] = _build()
    nc = _CACHE["nc"]

    res = run_bass_kernel_spmd(nc, _in_maps(x, W_q, W_k, W_v, W_proj),
                               core_ids=list(range(8)), trace=_trace)
    out = np.empty((B, S, D), dtype=np.float32)
    for b in range(B):
        acc = res.results[4 * b]["out_p"].astype(np.float32)
        for j in range(1, 4):
            acc = acc + res.results[4 * b + j]["out_p"]
        out[b] = acc + b_proj
    if _trace:
        _CACHE["last_trace"] = res
    return out
